# revision 21
# baseline (speedup 1.0000x reference)
"""Dual-stream attention kernel for TRN2 — one batch element per core (v19).

Per-core computation (batch element b):
  qb^T = Wq @ q_b^T          [C, N]   fp16, transposed layout (c on partitions)
  kb^T = Wk @ k_b^T          [C, N]   fp16
  vcomb[tb][tok, h, 0:64]   = (v_b @ Wv^T)    per-head slices   (natural layout)
  vcomb[tb][tok, h, 64:128] = (v_img_b @ Wvim^T)
  per head pair ct (2 heads = one 128-partition q/k tile), qh token half:
    S^T = kh @ qh^T        single-shot, 3-deep psum rotation s0/s1/s2
    E   = exp(S^T * scale) one 1024-wide ACTIVATE per kb, fp16, no max sub
    U   = [vh | vih]^T @ E accumulated over kb in the uvA psum pair
    r   = two DVE partial Esum chains (even/odd kb) + one merge add, then two
          single-shot M=1 ones-matmuls into the last score bank — no PE rowsum
          chains
    U is stashed unnormalized to SBUF (frees uv after one hop); 1/r via
    reciprocal_approx_fast per group; broadcast + multiply into uo are
    deferred by one group so the cross-engine tail hides behind the next
    group's scores/exps
  out^T = row-paired K=64 matmuls against wst (Wp rows on partitions 0:64,
    Wpi rows on 64:128, pre-permuted on host to u-channel order), accumulated
    over the 12 u-tiles; ACT Identity+bias drain, fp16 DMA out.

Weights are DMA'd once (hoisted before the For_i timing loop).  Stage-1
projection chains run pairwise interleaved across psum banks with drains
alternating ACT/DVE.  PSUM: four persistent [128, 2, 512] f32 tiles
(s0/s1/s2 scores + uvA), reused by stage 3 as the px/pxi accumulators.

Measured on HW (For_i slope): ~272 us/iter vs the 307 us baseline; engines
are latency-bound on cross-engine semaphore hops (~0.7 us each), not
PE-column-bound, which is why the deferred-normalize pipeline matters.

build_module(loop_n=N) wraps the body in a hardware For_i loop for wall-clock
timing; timing is data-independent.  stages may include "A"/"B" for timing
ablations (skip normalize / skip exp); the default "123" is the real kernel.
"""

import numpy as np
import concourse.bass as bass
import concourse.tile as tile
from concourse import bacc, mybir

P = 128
NTOK = 1024
C = 768
H = 12
DH = 64
CT = C // P  # 6 c-tiles
TB = NTOK // P  # 8 token blocks
QH = 2  # token halves
KB = 8  # k blocks
NQ = 512
NG = 12  # u-tile count (6 head-pairs x lo/up)
SCALE = DH**-0.5
F32 = mybir.dt.float32
F16 = mybir.dt.float16
EXP = mybir.ActivationFunctionType.Exp
IDENT = mybir.ActivationFunctionType.Identity
MULT = mybir.AluOpType.mult
ADD = mybir.AluOpType.add

XNAMES = ("xq", "xk", "xv", "xvi")


def build_module(num_devices=8, loop_n=1, stages="123"):
    nc = bacc.Bacc(
        "TRN2", target_bir_lowering=False, debug=False, num_devices=num_devices
    )
    d = {}
    for nm in XNAMES:
        d[nm] = nc.dram_tensor(nm, [C, NTOK], F16, kind="ExternalInput").ap()
    for nm in ("wq", "wk", "wv", "wvi"):
        d[nm] = nc.dram_tensor(nm, [C, C], F16, kind="ExternalInput").ap()
    d["wst"] = nc.dram_tensor("wst", [P, NG * CT * P], F16, kind="ExternalInput").ap()
    d["ones"] = nc.dram_tensor("ones", [P, KB], F16, kind="ExternalInput").ap()
    d["bp"] = nc.dram_tensor("bp", [P, CT], F32, kind="ExternalInput").ap()
    d["bpi"] = nc.dram_tensor("bpi", [P, CT], F32, kind="ExternalInput").ap()
    xo = nc.dram_tensor("xo", [C, NTOK], F16, kind="ExternalOutput").ap()
    xio = nc.dram_tensor("xio", [C, NTOK], F16, kind="ExternalOutput").ap()

    with tile.TileContext(nc) as tc:
        with (
            tc.tile_pool(name="persist", bufs=1) as pp,
            tc.tile_pool(name="xstage", bufs=2) as xpool,
            tc.tile_pool(name="e", bufs=8) as epool,
            tc.tile_pool(name="rp", bufs=4) as rppool,
            tc.tile_pool(name="rr", bufs=2) as rrpool,
            tc.tile_pool(name="ub", bufs=4) as ubpool,
            tc.tile_pool(name="ot", bufs=3) as opool,
            tc.tile_pool(name="pps", bufs=1, space="PSUM") as ppsum,
        ):
            qbt = pp.tile([P, CT, NTOK], F16, tag="qbt")
            kbt = pp.tile([P, CT, NTOK], F16, tag="kbt")
            vcomb = pp.tile([P, TB, H, P], F16, tag="vcomb")
            uo = pp.tile([P, NG, NTOK], F16, tag="uo")
            onest = pp.tile([P, KB], F16, tag="onest")
            bpr = pp.tile([P, CT], F32, tag="bpr")
            bpir = pp.tile([P, CT], F32, tag="bpir")
            w4 = pp.tile([P, 4, CT, C], F16, tag="w4")
            wst = pp.tile([P, NG, CT, P], F16, tag="wst")

            # all-8-banks psum: persistent tiles, manually scheduled
            s0 = ppsum.tile([P, 2, NQ], F32, tag="s0")
            s1 = ppsum.tile([P, 2, NQ], F32, tag="s1")
            s2 = ppsum.tile([P, 2, NQ], F32, tag="s2")
            uvA = ppsum.tile([P, 2, NQ], F32, tag="uvA")
            SLOTS = [
                s0[:, 0, :], s0[:, 1, :], s1[:, 0, :], s1[:, 1, :],
                s2[:, 0, :], s2[:, 1, :], uvA[:, 0, :], uvA[:, 1, :],
            ]

            # ---- hoisted: weights + constants (run once, before For_i) ----
            nc.sync.dma_start(bpr[:], d["bp"])
            nc.sync.dma_start(bpir[:], d["bpi"])
            nc.sync.dma_start(onest[:], d["ones"])
            for i, nm in enumerate(("wq", "wk", "wv", "wvi")):
                nc.sync.dma_start(
                    w4[:, i, :, :], d[nm].rearrange("(ct p) c -> p ct c", p=P)
                )
            nc.sync.dma_start(
                wst[:], d["wst"].rearrange("p (g ct k) -> p g ct k", g=NG, ct=CT)
            )

            def stage1():
                """Projections. Returns nothing; fills qbt/kbt/vcomb."""
                slot = [0]

                def next_slot():
                    s = SLOTS[slot[0] % 8]
                    slot[0] += 1
                    return s

                xts = {}
                for nm in XNAMES:
                    xt = xpool.tile([P, CT, NTOK], F16, tag="xt")
                    nc.sync.dma_start(
                        xt[:], d[nm].rearrange("(ct p) n -> p ct n", p=P)
                    )
                    xts[nm] = xt

                ndrain = [0]

                def drain(dst, src):
                    # alternate ACT/DVE so neither queue becomes the wall
                    if ndrain[0] % 2 == 0:
                        nc.scalar.copy(dst, src)
                    else:
                        nc.vector.tensor_copy(dst, src)
                    ndrain[0] += 1

                # q, k: transposed projections -> qbt/kbt
                # two chains (nh=0/1) interleaved so PSUM accumulate RMW
                # latency on one bank hides behind the other chain's stream
                for wi, (nm, dst) in enumerate((("xq", qbt), ("xk", kbt))):
                    xt = xts[nm]
                    for co in range(CT):
                        psa = next_slot()
                        psb = next_slot()
                        for ci in range(CT):
                            for nh, ps in ((0, psa), (1, psb)):
                                nc.tensor.matmul(
                                    ps,
                                    w4[:, wi, ci, co * P : (co + 1) * P],
                                    xt[:, ci, nh * NQ : (nh + 1) * NQ],
                                    start=(ci == 0),
                                    stop=(ci == CT - 1),
                                )
                        drain(dst[:, co, 0:NQ], psa)
                        drain(dst[:, co, NQ : 2 * NQ], psb)

                # v, v_img: natural projections -> vcomb head slices
                # interleave the v and v_img chains per (tb, strip)
                for tb in range(TB):
                    for si, (c0, cw) in enumerate(((0, 512), (512, 256))):
                        psa = next_slot()
                        psb = next_slot()
                        for ci in range(CT):
                            for wi, ps in ((2, psa), (3, psb)):
                                xt = xts["xv" if wi == 2 else "xvi"]
                                nc.tensor.matmul(
                                    ps[:, 0:cw],
                                    xt[:, ci, tb * P : (tb + 1) * P],
                                    w4[:, wi, ci, c0 : c0 + cw],
                                    start=(ci == 0),
                                    stop=(ci == CT - 1),
                                )
                        h0, h1 = c0 // DH, (c0 + cw) // DH
                        drain(
                            vcomb[:, tb, h0:h1, 0:DH],
                            psa[:, 0:cw].rearrange("p (h dh) -> p h dh", dh=DH),
                        )
                        drain(
                            vcomb[:, tb, h0:h1, DH:P],
                            psb[:, 0:cw].rearrange("p (h dh) -> p h dh", dh=DH),
                        )

            def stage2():
                no_norm = "A" in stages

                def normalize(item):
                    g_lo, g_up, qsl, ub, rr16 = item
                    rp_lo = rppool.tile([P, NQ], F16, tag="rp")
                    rp_up = rppool.tile([P, NQ], F16, tag="rp")
                    nc.gpsimd.partition_broadcast(rp_lo[:], rr16[0:1, 0, :])
                    nc.gpsimd.partition_broadcast(rp_up[:], rr16[0:1, 1, :])
                    nc.vector.tensor_tensor(
                        uo[:, g_lo, qsl], ub[:, 0, :], rp_lo[:], MULT
                    )
                    nc.vector.tensor_tensor(
                        uo[:, g_up, qsl], ub[:, 1, :], rp_up[:], MULT
                    )

                pending = []
                for ct in range(CT):
                    h_lo, h_up = 2 * ct, 2 * ct + 1
                    for qh in range(QH):
                        g_lo, g_up = 2 * ct, 2 * ct + 1
                        qsl = slice(qh * NQ, (qh + 1) * NQ)
                        uv = uvA
                        es = []
                        esumA = epool.tile([P, 2, NQ], F16, tag="esumA", bufs=2)
                        esumB = epool.tile([P, 2, NQ], F16, tag="esumB", bufs=2)
                        for kb in range(KB):
                            ksl = slice(kb * P, (kb + 1) * P)
                            s_blk = (s0, s1, s2)[kb % 3]
                            nc.tensor.matmul(
                                s_blk[:, 0, :], kbt[0:DH, ct, ksl],
                                qbt[0:DH, ct, qsl], start=True, stop=True,
                            )
                            nc.tensor.matmul(
                                s_blk[:, 1, :], kbt[DH:P, ct, ksl],
                                qbt[DH:P, ct, qsl], start=True, stop=True,
                            )
                            e_blk = epool.tile([P, 2, NQ], F16, tag="e", bufs=8)
                            nc.scalar.activation(e_blk[:], s_blk[:], EXP, scale=SCALE)
                            es.append(e_blk)
                            # running Esum on DVE: two independent partial
                            # chains (even/odd kb) halve the serial RAW depth
                            if kb == 2:
                                nc.vector.tensor_tensor(
                                    esumA[:], es[0][:], es[2][:], ADD
                                )
                            elif kb == 3:
                                nc.vector.tensor_tensor(
                                    esumB[:], es[1][:], es[3][:], ADD
                                )
                            elif kb >= 4:
                                t = esumA if kb % 2 == 0 else esumB
                                nc.vector.tensor_tensor(t[:], t[:], e_blk[:], ADD)
                            if kb > 0:
                                pe = es[kb - 1]
                                pkb = kb - 1
                                st, sp = pkb == 0, False
                                nc.tensor.matmul(
                                    uv[:, 0, :], vcomb[:, pkb, h_lo, :],
                                    pe[:, 0, :], start=st, stop=sp,
                                )
                                nc.tensor.matmul(
                                    uv[:, 1, :], vcomb[:, pkb, h_up, :],
                                    pe[:, 1, :], start=st, stop=sp,
                                )
                            if kb == 2 and pending:
                                # deferred normalize of the previous group:
                                # its tail hides behind this group's work
                                normalize(pending.pop())
                        pe = es[KB - 1]
                        nc.tensor.matmul(
                            uv[:, 0, :], vcomb[:, KB - 1, h_lo, :], pe[:, 0, :],
                            start=False, stop=True,
                        )
                        nc.tensor.matmul(
                            uv[:, 1, :], vcomb[:, KB - 1, h_up, :], pe[:, 1, :],
                            start=False, stop=True,
                        )
                        # stash unnormalized U (frees uv after one hop each)
                        ub = ubpool.tile([P, 2, NQ], F16, tag="ub", bufs=6)
                        nc.scalar.copy(ub[:, 0, :], uv[:, 0, :])
                        nc.vector.tensor_copy(ub[:, 1, :], uv[:, 1, :])
                        if no_norm:
                            nc.vector.tensor_copy(uo[:, g_lo, qsl], ub[:, 0, :])
                            nc.vector.tensor_copy(uo[:, g_up, qsl], ub[:, 1, :])
                            continue
                        # rowsums on the Esum; 1/r per group (fp16 rows)
                        nc.vector.tensor_tensor(esumA[:], esumA[:], esumB[:], ADD)
                        r_blk = (s0, s1, s2)[(KB - 1) % 3]
                        nc.tensor.matmul(
                            r_blk[0:1, 0, :], onest[:, 0:1], esumA[:, 0, :],
                            start=True, stop=True,
                        )
                        nc.tensor.matmul(
                            r_blk[0:1, 1, :], onest[:, 1:2], esumA[:, 1, :],
                            start=True, stop=True,
                        )
                        rr = rrpool.tile([1, 2, NQ], F32, tag="rr", bufs=2)
                        nc.vector.reciprocal_approx_fast(rr[:], r_blk[0:1, :, :])
                        rr16 = rrpool.tile([1, 2, NQ], F16, tag="rr16", bufs=3)
                        nc.vector.tensor_copy(rr16[:], rr[:])
                        pending.append((g_lo, g_up, qsl, ub, rr16))
                while pending:
                    normalize(pending.pop())

            def stage3():
                for co in range(CT):
                    if co % 2 == 0:
                        px, pxi = s0, s1
                    else:
                        px, pxi = s2, uvA
                    for th in range(2):
                        tsl = slice(th * NQ, (th + 1) * NQ)
                        for g in range(NG):
                            nc.tensor.matmul(
                                px[:, th, :],
                                wst[0:DH, g, co, :],
                                uo[0:DH, g, tsl],
                                start=(g == 0),
                                stop=(g == NG - 1),
                            )
                            nc.tensor.matmul(
                                pxi[:, th, :],
                                wst[DH:P, g, co, :],
                                uo[DH:P, g, tsl],
                                start=(g == 0),
                                stop=(g == NG - 1),
                            )
                    for th in range(2):
                        tsl = slice(th * NQ, (th + 1) * NQ)
                        for dst_dram, ps, bias_t in (
                            (xo, px, bpr),
                            (xio, pxi, bpir),
                        ):
                            ot = opool.tile([P, NQ], F16, tag="ot")
                            nc.scalar.activation(
                                ot[:], ps[:, th, :], IDENT,
                                bias=bias_t[:, co : co + 1], scale=1.0,
                            )
                            nc.sync.dma_start(
                                dst_dram[co * P : (co + 1) * P, tsl], ot[:]
                            )

            def body():
                if "1" in stages:
                    stage1()
                if "2" in stages:
                    stage2()
                if "3" in stages:
                    stage3()

            if loop_n == 1:
                body()
            else:
                with tc.For_i(0, loop_n, 1):
                    body()

    nc.compile()
    return nc


def make_in_maps(q, k, v, v_img, Wq, Wk, Wv, Wvim, Wp, bp, Wpi, bpi, n_cores=8):
    """Host-side prep: per-core transposed fp16 activations + shared fp16 weights."""
    f = np.float32
    h = np.float16
    wp = np.asarray(Wp, f).T.astype(h)  # [cin, cout]
    wpi = np.asarray(Wpi, f).T.astype(h)
    wst = np.zeros((P, NG, CT, P), h)
    for g in range(NG):
        hd = g  # u-tile g holds head g (g = 2*ct + half)
        rows = slice(DH * hd, DH * hd + DH)
        for co in range(CT):
            wst[0:DH, g, co, :] = wp[rows, co * P : (co + 1) * P]
            wst[DH:P, g, co, :] = wpi[rows, co * P : (co + 1) * P]
    shared = {
        "wq": np.asarray(Wq, f).T.astype(h),
        "wk": np.asarray(Wk, f).T.astype(h),
        "wv": np.asarray(Wv, f).T.astype(h),
        "wvi": np.asarray(Wvim, f).T.astype(h),
        "wst": np.ascontiguousarray(wst.reshape(P, NG * CT * P)),
        "ones": np.ones((P, KB), h),
        "bp": np.ascontiguousarray(np.asarray(bp, f).reshape(CT, P).T),
        "bpi": np.ascontiguousarray(np.asarray(bpi, f).reshape(CT, P).T),
    }
    q = np.asarray(q, f)
    k = np.asarray(k, f)
    v = np.asarray(v, f)
    vi = np.asarray(v_img, f)
    in_maps = []
    for b in range(n_cores):
        in_maps.append(
            {
                "xq": np.ascontiguousarray(q[:, b, :].T).astype(h),
                "xk": np.ascontiguousarray(k[:, b, :].T).astype(h),
                "xv": np.ascontiguousarray(v[:, b, :].T).astype(h),
                "xvi": np.ascontiguousarray(vi[:, b, :].T).astype(h),
                **shared,
            }
        )
    return in_maps


# ---------------------------------------------------------------------------
# Harness entry point: full inputs in, full outputs out.
# Shards batch B=8 across the 8 NeuronCores (data parallel), no collectives.
# ---------------------------------------------------------------------------

_NC_CACHE = {}


def _get_module():
    if "nc" not in _NC_CACHE:
        _NC_CACHE["nc"] = build_module(num_devices=8)
    return _NC_CACHE["nc"]


def kernel(q, k, v, v_img, Wq, Wk, Wv, Wvim, Wp, bp, Wpi, bpi):
    from concourse.bass_utils import run_bass_kernel_spmd

    B = np.asarray(q).shape[1]
    nc = _get_module()
    in_maps = make_in_maps(q, k, v, v_img, Wq, Wk, Wv, Wvim, Wp, bp, Wpi, bpi,
                           n_cores=B)
    res = run_bass_kernel_spmd(nc, in_maps, core_ids=list(range(B)), trace=False)
    x = np.stack([res.results[b]["xo"].T.astype(np.float32) for b in range(B)])
    x_im = np.stack([res.results[b]["xio"].T.astype(np.float32) for b in range(B)])
    return (x, x_im)


# revision 23
# speedup vs baseline: 1.0323x; 1.0323x over previous
"""Dual-stream attention kernel for TRN2 — one batch element per core (v19).

Per-core computation (batch element b):
  qb^T = Wq @ q_b^T          [C, N]   fp16, transposed layout (c on partitions)
  kb^T = Wk @ k_b^T          [C, N]   fp16
  vcomb[tb][tok, h, 0:64]   = (v_b @ Wv^T)    per-head slices   (natural layout)
  vcomb[tb][tok, h, 64:128] = (v_img_b @ Wvim^T)
  per head pair ct (2 heads = one 128-partition q/k tile), qh token half:
    S^T = kh @ qh^T        single-shot, 3-deep psum rotation s0/s1/s2
    E   = exp(S^T * scale) one 1024-wide ACTIVATE per kb, fp16, no max sub
    U   = [vh | vih]^T @ E accumulated over kb in the uvA psum pair
    r   = two DVE partial Esum chains (even/odd kb) + one merge add, then two
          single-shot M=1 ones-matmuls into the last score bank — no PE rowsum
          chains
    U is stashed unnormalized to SBUF (frees uv after one hop); 1/r via
    reciprocal_approx_fast per group; broadcast + multiply into uo are
    deferred by one group so the cross-engine tail hides behind the next
    group's scores/exps
  out^T = row-paired K=64 matmuls against wst (Wp rows on partitions 0:64,
    Wpi rows on 64:128, pre-permuted on host to u-channel order), accumulated
    over the 12 u-tiles; ACT Identity+bias drain, fp16 DMA out.

Weights are DMA'd once (hoisted before the For_i timing loop).  Stage-1
projection chains run pairwise interleaved across psum banks with drains
alternating ACT/DVE.  PSUM: four persistent [128, 2, 512] f32 tiles
(s0/s1/s2 scores + uvA), reused by stage 3 as the px/pxi accumulators.

Measured on HW (For_i slope): ~272 us/iter vs the 307 us baseline; engines
are latency-bound on cross-engine semaphore hops (~0.7 us each), not
PE-column-bound, which is why the deferred-normalize pipeline matters.

build_module(loop_n=N) wraps the body in a hardware For_i loop for wall-clock
timing; timing is data-independent.  stages may include "A"/"B" for timing
ablations (skip normalize / skip exp); the default "123" is the real kernel.
"""

import numpy as np
import concourse.bass as bass
import concourse.tile as tile
from concourse import bacc, mybir

P = 128
NTOK = 1024
C = 768
H = 12
DH = 64
CT = C // P  # 6 c-tiles
TB = NTOK // P  # 8 token blocks
QH = 2  # token halves
KB = 8  # k blocks
NQ = 512
NG = 12  # u-tile count (6 head-pairs x lo/up)
SCALE = DH**-0.5
F32 = mybir.dt.float32
F16 = mybir.dt.float16
EXP = mybir.ActivationFunctionType.Exp
IDENT = mybir.ActivationFunctionType.Identity
MULT = mybir.AluOpType.mult
ADD = mybir.AluOpType.add

XNAMES = ("xq", "xk", "xv", "xvi")


def build_module(num_devices=8, loop_n=1, stages="123"):
    nc = bacc.Bacc(
        "TRN2", target_bir_lowering=False, debug=False, num_devices=num_devices
    )
    d = {}
    for nm in XNAMES:
        d[nm] = nc.dram_tensor(nm, [C, NTOK], F16, kind="ExternalInput").ap()
    for nm in ("wq", "wk", "wv", "wvi"):
        d[nm] = nc.dram_tensor(nm, [C, C], F16, kind="ExternalInput").ap()
    d["wst"] = nc.dram_tensor("wst", [P, NG * CT * P], F16, kind="ExternalInput").ap()
    d["ones"] = nc.dram_tensor("ones", [P, KB], F16, kind="ExternalInput").ap()
    d["bp"] = nc.dram_tensor("bp", [P, CT], F32, kind="ExternalInput").ap()
    d["bpi"] = nc.dram_tensor("bpi", [P, CT], F32, kind="ExternalInput").ap()
    xo = nc.dram_tensor("xo", [C, NTOK], F16, kind="ExternalOutput").ap()
    xio = nc.dram_tensor("xio", [C, NTOK], F16, kind="ExternalOutput").ap()

    with tile.TileContext(nc) as tc:
        with (
            tc.tile_pool(name="persist", bufs=1) as pp,
            tc.tile_pool(name="xstage", bufs=2) as xpool,
            tc.tile_pool(name="e", bufs=8) as epool,
            tc.tile_pool(name="rp", bufs=4) as rppool,
            tc.tile_pool(name="rr", bufs=2) as rrpool,
            tc.tile_pool(name="ub", bufs=4) as ubpool,
            tc.tile_pool(name="ot", bufs=3) as opool,
            tc.tile_pool(name="pps", bufs=1, space="PSUM") as ppsum,
        ):
            qbt = pp.tile([P, CT, NTOK], F16, tag="qbt")
            kbt = pp.tile([P, CT, NTOK], F16, tag="kbt")
            vcomb = pp.tile([P, TB, H, P], F16, tag="vcomb")
            uo = pp.tile([P, NG, NTOK], F16, tag="uo")
            onest = pp.tile([P, KB], F16, tag="onest")
            bpr = pp.tile([P, CT], F32, tag="bpr")
            bpir = pp.tile([P, CT], F32, tag="bpir")
            w4 = pp.tile([P, 4, CT, C], F16, tag="w4")
            wst = pp.tile([P, NG, CT, P], F16, tag="wst")

            # all-8-banks psum: persistent tiles, manually scheduled
            s0 = ppsum.tile([P, 2, NQ], F32, tag="s0")
            s1 = ppsum.tile([P, 2, NQ], F32, tag="s1")
            s2 = ppsum.tile([P, 2, NQ], F32, tag="s2")
            uvA = ppsum.tile([P, 2, NQ], F32, tag="uvA")
            SLOTS = [
                s0[:, 0, :], s0[:, 1, :], s1[:, 0, :], s1[:, 1, :],
                s2[:, 0, :], s2[:, 1, :], uvA[:, 0, :], uvA[:, 1, :],
            ]

            # ---- hoisted: weights + constants (run once, before For_i) ----
            nc.sync.dma_start(bpr[:], d["bp"])
            nc.sync.dma_start(bpir[:], d["bpi"])
            nc.sync.dma_start(onest[:], d["ones"])
            for i, nm in enumerate(("wq", "wk", "wv", "wvi")):
                nc.sync.dma_start(
                    w4[:, i, :, :], d[nm].rearrange("(ct p) c -> p ct c", p=P)
                )
            nc.sync.dma_start(
                wst[:], d["wst"].rearrange("p (g ct k) -> p g ct k", g=NG, ct=CT)
            )

            def stage1(part="qk", xts=None):
                """Projections; part='qk' emits q/k (returns xts), 'vvi' the rest."""
                slot = [0]

                def next_slot():
                    s = SLOTS[slot[0] % 8]
                    slot[0] += 1
                    return s

                if xts is None:
                    xts = {}
                    for nm in XNAMES:
                        xt = xpool.tile([P, CT, NTOK], F16, tag="xt")
                        nc.sync.dma_start(
                            xt[:], d[nm].rearrange("(ct p) n -> p ct n", p=P)
                        )
                        xts[nm] = xt

                ndrain = [0]

                def drain(dst, src):
                    # alternate ACT/DVE so neither queue becomes the wall
                    if ndrain[0] % 2 == 0:
                        nc.scalar.copy(dst, src)
                    else:
                        nc.vector.tensor_copy(dst, src)
                    ndrain[0] += 1

                # q, k: transposed projections -> qbt/kbt
                # two chains (nh=0/1) interleaved so PSUM accumulate RMW
                # latency on one bank hides behind the other chain's stream
                for wi, (nm, dst) in enumerate((("xq", qbt), ("xk", kbt))):
                    if part != "qk":
                        break
                    xt = xts[nm]
                    for co in range(CT):
                        psa = next_slot()
                        psb = next_slot()
                        for ci in range(CT):
                            for nh, ps in ((0, psa), (1, psb)):
                                nc.tensor.matmul(
                                    ps,
                                    w4[:, wi, ci, co * P : (co + 1) * P],
                                    xt[:, ci, nh * NQ : (nh + 1) * NQ],
                                    start=(ci == 0),
                                    stop=(ci == CT - 1),
                                )
                        drain(dst[:, co, 0:NQ], psa)
                        drain(dst[:, co, NQ : 2 * NQ], psb)

                if part == "qk":
                    return xts
                # v, v_img: natural projections -> vcomb head slices
                # interleave the v and v_img chains per (tb, strip)
                for tb in range(TB):
                    for si, (c0, cw) in enumerate(((0, 512), (512, 256))):
                        psa = next_slot()
                        psb = next_slot()
                        for ci in range(CT):
                            for wi, ps in ((2, psa), (3, psb)):
                                xt = xts["xv" if wi == 2 else "xvi"]
                                nc.tensor.matmul(
                                    ps[:, 0:cw],
                                    xt[:, ci, tb * P : (tb + 1) * P],
                                    w4[:, wi, ci, c0 : c0 + cw],
                                    start=(ci == 0),
                                    stop=(ci == CT - 1),
                                )
                        h0, h1 = c0 // DH, (c0 + cw) // DH
                        drain(
                            vcomb[:, tb, h0:h1, 0:DH],
                            psa[:, 0:cw].rearrange("p (h dh) -> p h dh", dh=DH),
                        )
                        drain(
                            vcomb[:, tb, h0:h1, DH:P],
                            psb[:, 0:cw].rearrange("p (h dh) -> p h dh", dh=DH),
                        )

            def emit_scores(ct, qsl):
                """Scores + exp + Esum partials for one group; returns state."""
                es = []
                esumA = epool.tile([P, 2, NQ], F16, tag="esumA", bufs=2)
                esumB = epool.tile([P, 2, NQ], F16, tag="esumB", bufs=2)
                for kb in range(KB):
                    ksl = slice(kb * P, (kb + 1) * P)
                    s_blk = (s0, s1, s2)[kb % 3]
                    nc.tensor.matmul(
                        s_blk[:, 0, :], kbt[0:DH, ct, ksl],
                        qbt[0:DH, ct, qsl], start=True, stop=True,
                    )
                    nc.tensor.matmul(
                        s_blk[:, 1, :], kbt[DH:P, ct, ksl],
                        qbt[DH:P, ct, qsl], start=True, stop=True,
                    )
                    e_blk = epool.tile([P, 2, NQ], F16, tag="e", bufs=8)
                    nc.scalar.activation(e_blk[:], s_blk[:], EXP, scale=SCALE)
                    es.append(e_blk)
                    if kb == 2:
                        nc.vector.tensor_tensor(esumA[:], es[0][:], es[2][:], ADD)
                    elif kb == 3:
                        nc.vector.tensor_tensor(esumB[:], es[1][:], es[3][:], ADD)
                    elif kb >= 4:
                        t = esumA if kb % 2 == 0 else esumB
                        nc.vector.tensor_tensor(t[:], t[:], e_blk[:], ADD)
                return es, esumA, esumB

            def stage2(pre=None):
                no_norm = "A" in stages

                def normalize(item):
                    g_lo, g_up, qsl, ub, rr16 = item
                    rp_lo = rppool.tile([P, NQ], F16, tag="rp")
                    rp_up = rppool.tile([P, NQ], F16, tag="rp")
                    nc.gpsimd.partition_broadcast(rp_lo[:], rr16[0:1, 0, :])
                    nc.gpsimd.partition_broadcast(rp_up[:], rr16[0:1, 1, :])
                    nc.vector.tensor_tensor(
                        uo[:, g_lo, qsl], ub[:, 0, :], rp_lo[:], MULT
                    )
                    nc.vector.tensor_tensor(
                        uo[:, g_up, qsl], ub[:, 1, :], rp_up[:], MULT
                    )

                pending = []
                first = True
                for ct in range(CT):
                    h_lo, h_up = 2 * ct, 2 * ct + 1
                    for qh in range(QH):
                        g_lo, g_up = 2 * ct, 2 * ct + 1
                        qsl = slice(qh * NQ, (qh + 1) * NQ)
                        uv = uvA
                        if first and pre is not None:
                            es, esumA, esumB = pre
                        else:
                            es, esumA, esumB = emit_scores(ct, qsl)
                        first = False
                        for kb in range(KB):
                            if kb > 0:
                                pe = es[kb - 1]
                                pkb = kb - 1
                                st, sp = pkb == 0, False
                                nc.tensor.matmul(
                                    uv[:, 0, :], vcomb[:, pkb, h_lo, :],
                                    pe[:, 0, :], start=st, stop=sp,
                                )
                                nc.tensor.matmul(
                                    uv[:, 1, :], vcomb[:, pkb, h_up, :],
                                    pe[:, 1, :], start=st, stop=sp,
                                )
                            if kb == 2 and pending:
                                # deferred normalize of the previous group:
                                # its tail hides behind this group's work
                                normalize(pending.pop())
                        pe = es[KB - 1]
                        nc.tensor.matmul(
                            uv[:, 0, :], vcomb[:, KB - 1, h_lo, :], pe[:, 0, :],
                            start=False, stop=True,
                        )
                        nc.tensor.matmul(
                            uv[:, 1, :], vcomb[:, KB - 1, h_up, :], pe[:, 1, :],
                            start=False, stop=True,
                        )
                        # stash unnormalized U (frees uv after one hop each)
                        ub = ubpool.tile([P, 2, NQ], F16, tag="ub", bufs=6)
                        nc.scalar.copy(ub[:, 0, :], uv[:, 0, :])
                        nc.vector.tensor_copy(ub[:, 1, :], uv[:, 1, :])
                        if no_norm:
                            nc.vector.tensor_copy(uo[:, g_lo, qsl], ub[:, 0, :])
                            nc.vector.tensor_copy(uo[:, g_up, qsl], ub[:, 1, :])
                            continue
                        # rowsums on the Esum; 1/r per group (fp16 rows)
                        nc.vector.tensor_tensor(esumA[:], esumA[:], esumB[:], ADD)
                        r_blk = (s0, s1, s2)[(KB - 1) % 3]
                        nc.tensor.matmul(
                            r_blk[0:1, 0, :], onest[:, 0:1], esumA[:, 0, :],
                            start=True, stop=True,
                        )
                        nc.tensor.matmul(
                            r_blk[0:1, 1, :], onest[:, 1:2], esumA[:, 1, :],
                            start=True, stop=True,
                        )
                        rr = rrpool.tile([1, 2, NQ], F32, tag="rr", bufs=2)
                        nc.vector.reciprocal_approx_fast(rr[:], r_blk[0:1, :, :])
                        rr16 = rrpool.tile([1, 2, NQ], F16, tag="rr16", bufs=3)
                        nc.vector.tensor_copy(rr16[:], rr[:])
                        pending.append((g_lo, g_up, qsl, ub, rr16))
                while pending:
                    normalize(pending.pop())

            def stage3():
                for co in range(CT):
                    if co % 2 == 0:
                        px, pxi = s0, s1
                    else:
                        px, pxi = s2, uvA
                    for th in range(2):
                        tsl = slice(th * NQ, (th + 1) * NQ)
                        for g in range(NG):
                            nc.tensor.matmul(
                                px[:, th, :],
                                wst[0:DH, g, co, :],
                                uo[0:DH, g, tsl],
                                start=(g == 0),
                                stop=(g == NG - 1),
                            )
                            nc.tensor.matmul(
                                pxi[:, th, :],
                                wst[DH:P, g, co, :],
                                uo[DH:P, g, tsl],
                                start=(g == 0),
                                stop=(g == NG - 1),
                            )
                    for th in range(2):
                        tsl = slice(th * NQ, (th + 1) * NQ)
                        for dst_dram, ps, bias_t in (
                            (xo, px, bpr),
                            (xio, pxi, bpir),
                        ):
                            ot = opool.tile([P, NQ], F16, tag="ot")
                            nc.scalar.activation(
                                ot[:], ps[:, th, :], IDENT,
                                bias=bias_t[:, co : co + 1], scale=1.0,
                            )
                            nc.sync.dma_start(
                                dst_dram[co * P : (co + 1) * P, tsl], ot[:]
                            )

            def body():
                pre = None
                if "1" in stages:
                    xts = stage1("qk")
                    if "2" in stages:
                        # prefetch group-0 scores/exps: ACT works during the
                        # v/v_img projections instead of idling
                        pre = emit_scores(0, slice(0, NQ))
                    stage1("vvi", xts)
                if "2" in stages:
                    stage2(pre)
                if "3" in stages:
                    stage3()

            if loop_n == 1:
                body()
            else:
                with tc.For_i(0, loop_n, 1):
                    body()

    nc.compile()
    return nc


def make_in_maps(q, k, v, v_img, Wq, Wk, Wv, Wvim, Wp, bp, Wpi, bpi, n_cores=8):
    """Host-side prep: per-core transposed fp16 activations + shared fp16 weights."""
    f = np.float32
    h = np.float16
    wp = np.asarray(Wp, f).T.astype(h)  # [cin, cout]
    wpi = np.asarray(Wpi, f).T.astype(h)
    wst = np.zeros((P, NG, CT, P), h)
    for g in range(NG):
        hd = g  # u-tile g holds head g (g = 2*ct + half)
        rows = slice(DH * hd, DH * hd + DH)
        for co in range(CT):
            wst[0:DH, g, co, :] = wp[rows, co * P : (co + 1) * P]
            wst[DH:P, g, co, :] = wpi[rows, co * P : (co + 1) * P]
    shared = {
        "wq": np.asarray(Wq, f).T.astype(h),
        "wk": np.asarray(Wk, f).T.astype(h),
        "wv": np.asarray(Wv, f).T.astype(h),
        "wvi": np.asarray(Wvim, f).T.astype(h),
        "wst": np.ascontiguousarray(wst.reshape(P, NG * CT * P)),
        "ones": np.ones((P, KB), h),
        "bp": np.ascontiguousarray(np.asarray(bp, f).reshape(CT, P).T),
        "bpi": np.ascontiguousarray(np.asarray(bpi, f).reshape(CT, P).T),
    }
    q = np.asarray(q, f)
    k = np.asarray(k, f)
    v = np.asarray(v, f)
    vi = np.asarray(v_img, f)
    in_maps = []
    for b in range(n_cores):
        in_maps.append(
            {
                "xq": np.ascontiguousarray(q[:, b, :].T).astype(h),
                "xk": np.ascontiguousarray(k[:, b, :].T).astype(h),
                "xv": np.ascontiguousarray(v[:, b, :].T).astype(h),
                "xvi": np.ascontiguousarray(vi[:, b, :].T).astype(h),
                **shared,
            }
        )
    return in_maps


# ---------------------------------------------------------------------------
# Harness entry point: full inputs in, full outputs out.
# Shards batch B=8 across the 8 NeuronCores (data parallel), no collectives.
# ---------------------------------------------------------------------------

_NC_CACHE = {}


def _get_module():
    if "nc" not in _NC_CACHE:
        _NC_CACHE["nc"] = build_module(num_devices=8)
    return _NC_CACHE["nc"]


def kernel(q, k, v, v_img, Wq, Wk, Wv, Wvim, Wp, bp, Wpi, bpi):
    from concourse.bass_utils import run_bass_kernel_spmd

    B = np.asarray(q).shape[1]
    nc = _get_module()
    in_maps = make_in_maps(q, k, v, v_img, Wq, Wk, Wv, Wvim, Wp, bp, Wpi, bpi,
                           n_cores=B)
    res = run_bass_kernel_spmd(nc, in_maps, core_ids=list(range(B)), trace=False)
    x = np.stack([res.results[b]["xo"].T.astype(np.float32) for b in range(B)])
    x_im = np.stack([res.results[b]["xio"].T.astype(np.float32) for b in range(B)])
    return (x, x_im)
